# revision 19
# baseline (speedup 1.0000x reference)
"""Trainium2 Bass kernel for nn_LinearEncoder (gnn_message_passing).

Reference computes, for N=512 nodes with n_in = n_out = 256:
    i, j = triu_indices(N, k=1)
    edges = concat([x[i], x[j]], -1)            # [E, 512]
    h = edges @ W.T + b                         # [E, n_out]
    out[i, j] = h ; out = out + out.T           # [N, N, 256], 0 diagonal

Key identities: with W = [W1 | W2], A = x @ W1.T, B' = x @ W2.T + b,
the full output is symmetric with zero diagonal and, on the upper
triangle (i < j),
    out[i, j, c] = A[i, c] + B'[j, c].
The device therefore only materialises the strict upper triangle; the
host's unshard step places each value at both (i, j) and (j, i) (the
diagonal stays at the scatter-init zero), halving the HBM write stream
versus a full-matrix kernel.

Layout: channels on SBUF partitions (two 128-channel halves), nodes on
the free dimension.  The column tables B'T[c, j] fall straight out of
two K=256 matmuls against the uploaded x.T; the per-row terms A[i, c]
are [128, 1] columns of RS = W1 @ x[rows_k].T (host slices the 64
owned rows per core), so each output row segment is a single
per-partition-scalar add: out_seg = B'T[:, j0:512] + RS[:, m].

Sharding: core k owns rows i = 32*b + 4*k + v (b in [0,16), v in
[0,4)) — four rows from every 32-row column block, so each core's
upper-triangle rectangles (cols [32b, 512) for block b) have identical
shapes across cores (one SPMD program) and identical total bytes.
Blocks b and 15-b pair into eight slabs (~8.9 MB/core total): pairs
p=0,1 ship as two-row [128, 2176] half-slab tiles (544 KB DMAs whose
per-tile dependency tracking keeps the Tile scheduler's first-DMA wait
thresholds tight, starting the write stream ~16.6 us in), while pairs
p=2..7 ship as full [128, 4352] tiles (1.09 MB DMAs for best sustained
HBM write bandwidth, ~400 GB/s measured).  Sub-diagonal lanes inside a
rectangle are shipped as garbage and discarded by the host (the mirror
of the transposed upper triangle supplies those entries).

Engine assignment is calibrated to measured TRN2 op costs: DVE
tensor_scalar (4x uop, ~165 ns overhead + 0.26 ns/col) takes the wide
segments plus fused block-B adds for p=3..5, ScalarE ACT-with-bias
(~187 + 0.83/col) the mid segments plus per-v block-B for p=0,6,7, and
GpSimd only the two narrowest fused block-B pairs (its broadcast
tensor_tensor steals SBUF bandwidth from concurrent DVE ops, so its
share is kept small).  GpSimd tensor_scalar is never used (7x slower
than its tensor_tensor).  Fused ops add the RS16 row column via
stride-0/stride-1 broadcast APs over two v rows at once.
"""

import os
import sys

for _p in ("/opt/trn_rl_repo", "/root/.axon_site/_ro/trn_rl_repo"):
    if os.path.isdir(_p) and _p not in sys.path:
        sys.path.insert(0, _p)

import numpy as np
import ml_dtypes

import concourse.bass as bass
import concourse.bacc as bacc
import concourse.mybir as mybir
import concourse.tile as tile
from concourse.bass_utils import run_bass_kernel_spmd

N = 512
CH = 256          # n_out
NIN = 256         # n_in
NCORES = 8
NB = 16           # column blocks of 32
RPB = 4           # rows per block per core
F32 = mybir.dt.float32
BF16 = mybir.dt.bfloat16
BF16NP = ml_dtypes.bfloat16

SEG = 1088        # per-v columns: 2*(w1 + w2), w1 + w2 = 544
HSLABW = 2 * SEG  # 2176, half-slab = two v rows
WXW = 2 * CH + 64 + 4  # w12 half | xsel half | b bits = 580


def _rows_for_core(k: int) -> np.ndarray:
    """Row m = 4*b + v owns global row 32*b + 4*k + v."""
    b = np.repeat(np.arange(NB), RPB)
    v = np.tile(np.arange(RPB), NB)
    return 32 * b + RPB * k + v


# --------------------------------------------------------------------------
# device program
# --------------------------------------------------------------------------

_PROGRAM = None


def _build_program() -> bass.Bass:
    nc = bacc.Bacc()
    ADD = mybir.AluOpType.add

    # wx[h]: [w12t[128h:128h+128] | xselt[128h:..] | bbits (h=0)]
    wx = nc.dram_tensor("wx", [2, 128, WXW], BF16, kind="ExternalInput")
    xt = nc.dram_tensor("xt", [NIN, N], BF16, kind="ExternalInput")
    # hslab[2p + hf]: blocks (p, 15-p), v rows {2hf, 2hf+1}; per local
    # u in [0,2): [A h0 (w1) | A h1 (w1) | B h0 (w2) | B h1 (w2)] at
    # offset 1088*u, w1 = 512-32p, w2 = 32+32p.
    hslab = nc.dram_tensor("hslab", [16, 128, HSLABW], BF16,
                           kind="ExternalOutput")

    with tile.TileContext(nc) as tc:
        with (
            tc.tile_pool(name="const", bufs=1) as cpool,
            tc.tile_pool(name="psB", bufs=2, space="PSUM") as psB,
            tc.tile_pool(name="psR", bufs=2, space="PSUM") as psR,
            tc.tile_pool(name="slabs", bufs=8) as spool,
        ):
            # ---- input loads: four DMAs on sync, matmul deps first -------
            WX, XT = [], []
            for h in range(2):
                t = cpool.tile([128, WXW], BF16, tag=f"wx{h}")
                nc.sync.dma_start(out=t[:], in_=wx[h])
                WX.append(t)
            for h in range(2):
                t = cpool.tile([128, N], BF16, tag=f"xt{h}")
                nc.sync.dma_start(out=t[:], in_=xt[128 * h:128 * (h + 1), :])
                XT.append(t)
            XS = [WX[h][:, 2 * CH:2 * CH + 64] for h in range(2)]
            bc = WX[0][:, 2 * CH + 64:WXW].bitcast(F32)  # [128, 2] f32

            mm = nc.tensor.matmul

            # ---- row terms RS[c, m] = A[row_m, c], two halves ------------
            RS = [None, None]    # f32, scalar operands for TS / ACT bias
            RS16 = [None, None]  # bf16, in1 for fused tensor_tensor
            for h in range(2):
                pr = psR.tile([128, 64], F32, tag="pr", name=f"pr{h}")
                lo = 128 * h  # W1.T columns
                mm(pr[:], WX[0][:, lo:lo + 128], XS[0],
                   start=True, stop=False)
                mm(pr[:], WX[1][:, lo:lo + 128], XS[1],
                   start=False, stop=True)
                rs = cpool.tile([128, 64], F32, tag=f"RS{h}")
                rs16 = cpool.tile([128, 64], BF16, tag=f"RS16{h}")
                nc.vector.tensor_copy(out=rs[:], in_=pr[:])
                nc.scalar.copy(out=rs16[:], in_=pr[:])
                RS[h] = rs
                RS16[h] = rs16

            # ---- column tables B'T[c, j] = B[j, c] + b[c], two halves ----
            BT = [None, None]
            for h in range(2):
                pb = psB.tile([128, N], F32, tag="pb", name=f"pb{h}")
                lo = CH + 128 * h  # W2.T columns
                mm(pb[:], WX[0][:, lo:lo + 128], XT[0][:],
                   start=True, stop=False)
                mm(pb[:], WX[1][:, lo:lo + 128], XT[1][:],
                   start=False, stop=True)
                bt = cpool.tile([128, N], BF16, tag=f"BT{h}")
                if h == 0:
                    nc.vector.tensor_scalar_add(bt[:], pb[:], bc[:, 0:1])
                else:
                    nc.scalar.add(bt[:], pb[:], bc[:, 1:2])
                BT[h] = bt

            def fused_B(eng, S, p, h, w1, w2, cB, v0, nv):
                """One op for block-B half h over nv rows starting at v0."""
                sfull = S[:]
                out = bass.AP(sfull.tensor, sfull.offset + 2 * w1 + h * w2,
                              [sfull.ap[0], [SEG, nv], [1, w2]])
                btf = BT[h][:]
                in0 = bass.AP(btf.tensor, btf.offset + cB,
                              [btf.ap[0], [0, nv], [1, w2]])
                rsf = RS16[h][:]
                in1 = bass.AP(rsf.tensor,
                              rsf.offset + RPB * (15 - p) + v0,
                              [rsf.ap[0], [1, nv], [0, w2]])
                eng.tensor_tensor(out=out, in0=in0, in1=in1, op=ADD)

            # ---- main loop: p<=1 ship as two half-slabs (fast, tightly
            # tracked stream start), p>=2 as one full slab (DMA efficiency).
            for p in range(8):
                w1 = N - 32 * p          # block p rect width (cols 32p..512)
                w2 = 32 + 32 * p         # block 15-p width
                cA = 32 * p              # B'T col offset for block p
                cB = N - w2              # for block 15-p
                halves = 2 if p <= 1 else 1
                nv = 2 if p <= 1 else RPB
                for hf in range(halves):
                    S = spool.tile([128, nv * SEG], BF16,
                                   tag=("sh" if p <= 1 else "sf"),
                                   name=f"s{p}_{hf}")
                    for u in range(nv):
                        v = nv * hf + u
                        off = SEG * u
                        mA = RPB * p + v
                        mB = RPB * (15 - p) + v
                        sA0 = S[:, off:off + w1]
                        sA1 = S[:, off + w1:off + 2 * w1]
                        nc.vector.tensor_scalar_add(
                            sA0, BT[0][:, cA:N], RS[0][:, mA:mA + 1])
                        if p == 0 or p >= 4:
                            nc.vector.tensor_scalar_add(
                                sA1, BT[1][:, cA:N], RS[1][:, mA:mA + 1])
                        else:
                            nc.scalar.add(sA1, BT[1][:, cA:N],
                                          RS[1][:, mA:mA + 1])
                        if p == 0 or p >= 6:
                            for h in range(2):
                                sB = S[:, off + 2 * w1 + h * w2:
                                       off + 2 * w1 + (h + 1) * w2]
                                nc.scalar.add(sB, BT[h][:, cB:N],
                                              RS[h][:, mB:mB + 1])
                    if 1 <= p <= 2:
                        fused_B(nc.gpsimd, S, p, 0, w1, w2, cB, nv * hf, nv)
                        fused_B(nc.gpsimd, S, p, 1, w1, w2, cB, nv * hf, nv)
                    elif 3 <= p <= 5:
                        fused_B(nc.vector, S, p, 0, w1, w2, cB, nv * hf, nv)
                        fused_B(nc.vector, S, p, 1, w1, w2, cB, nv * hf, nv)
                    if p <= 1:
                        nc.sync.dma_start(out=hslab[2 * p + hf], in_=S[:])
                    else:
                        h0 = hslab[2 * p]
                        out_ap = bass.AP(
                            h0.tensor, h0.offset,
                            [h0.ap[0], [128 * HSLABW, 2], [1, HSLABW]])
                        q = nc.sync if p % 2 == 0 else nc.scalar
                        q.dma_start(out=out_ap, in_=S[:])

    nc.compile()
    return nc


def _program() -> bass.Bass:
    global _PROGRAM
    if _PROGRAM is None:
        _PROGRAM = _build_program()
    return _PROGRAM


# --------------------------------------------------------------------------
# host entry point
# --------------------------------------------------------------------------

def build_in_maps(x, W, b):
    x = np.asarray(x, np.float32)
    W = np.asarray(W, np.float32)
    b = np.asarray(b, np.float32)
    w12 = np.concatenate(
        [np.ascontiguousarray(W[:, :NIN].T),
         np.ascontiguousarray(W[:, NIN:].T)], axis=1)  # [in, 512]
    xtf = np.ascontiguousarray(x.T).astype(BF16NP)
    w12 = w12.astype(BF16NP)
    bbits = np.ascontiguousarray(
        np.stack([b[0:128], b[128:256]], axis=1)).view(BF16NP)  # [128, 4]
    maps = []
    for k in range(NCORES):
        rows = _rows_for_core(k)
        xsel = np.ascontiguousarray(x[rows].T).astype(BF16NP)
        wx = np.zeros((2, 128, WXW), BF16NP)
        for h in range(2):
            lo = 128 * h
            wx[h, :, 0:2 * CH] = w12[lo:lo + 128]
            wx[h, :, 2 * CH:2 * CH + 64] = xsel[lo:lo + 128]
        wx[0, :, 2 * CH + 64:WXW] = bbits
        maps.append({"wx": wx, "xt": xtf})
    return maps


def _assemble(results):
    """8 per-core half-slab dicts -> full [512, 512, 256] f32 output."""
    out = np.zeros((N, N, CH), np.float32)
    ar = np.arange(2)
    for k in range(NCORES):
        hs = np.asarray(results[k]["hslab"]).astype(np.float32)
        for p in range(8):
            w1 = N - 32 * p
            w2 = 32 + 32 * p
            for hf in range(2):
                sp = hs[2 * p + hf].reshape(128, 2, SEG)
                rowsA = 32 * p + RPB * k + 2 * hf + ar
                rowsB = 32 * (15 - p) + RPB * k + 2 * hf + ar
                out[rowsA, 32 * p:N, 0:128] = \
                    sp[:, :, 0:w1].transpose(1, 2, 0)
                out[rowsA, 32 * p:N, 128:256] = \
                    sp[:, :, w1:2 * w1].transpose(1, 2, 0)
                out[rowsB, N - w2:N, 0:128] = \
                    sp[:, :, 2 * w1:2 * w1 + w2].transpose(1, 2, 0)
                out[rowsB, N - w2:N, 128:256] = \
                    sp[:, :, 2 * w1 + w2:SEG].transpose(1, 2, 0)
    # unshard: keep the strict upper triangle (sub-diagonal rect lanes are
    # garbage), mirror it across the diagonal; diag stays scatter-init 0.
    tril = np.tril_indices(N)
    out[tril] = 0.0
    return out + out.transpose(1, 0, 2)


def kernel(x, W, b):
    nc = _program()
    in_maps = build_in_maps(x, W, b)
    res = run_bass_kernel_spmd(nc, in_maps, core_ids=list(range(NCORES)))
    return _assemble(res.results)


# revision 20
# speedup vs baseline: 1.1041x; 1.1041x over previous
"""Trainium2 Bass kernel for nn_LinearEncoder (gnn_message_passing).

Reference computes, for N=512 nodes with n_in = n_out = 256:
    i, j = triu_indices(N, k=1)
    edges = concat([x[i], x[j]], -1)            # [E, 512]
    h = edges @ W.T + b                         # [E, n_out]
    out[i, j] = h ; out = out + out.T           # [N, N, 256], 0 diagonal

Key identities: with W = [W1 | W2], A = x @ W1.T, B' = x @ W2.T + b,
the full output is symmetric with zero diagonal and, on the upper
triangle (i < j),
    out[i, j, c] = A[i, c] + B'[j, c].
The device therefore only materialises the strict upper triangle; the
host's unshard step places each value at both (i, j) and (j, i) (the
diagonal stays at the scatter-init zero), halving the HBM write stream
versus a full-matrix kernel.

Layout: channels on SBUF partitions (two 128-channel halves), nodes on
the free dimension.  The column tables B'T[c, j] fall straight out of
two K=256 matmuls against the uploaded x.T; the per-row terms A[i, c]
are [128, 1] columns of RS = W1 @ x[rows_k].T (host slices the 64
owned rows per core), so each output row segment is a single
per-partition-scalar add: out_seg = B'T[:, j0:512] + RS[:, m].

Sharding: core k owns rows i = 32*b + 4*k + v (b in [0,16), v in
[0,4)) — four rows from every 32-row column block, so each core's
upper-triangle rectangles (cols [32b, 512) for block b) have identical
shapes across cores (one SPMD program) and identical total bytes.
Blocks b and 15-b pair into eight slabs (~8.9 MB/core total): pairs
p=0,1 ship as two-row [128, 2176] half-slab tiles (544 KB DMAs whose
per-tile dependency tracking keeps the Tile scheduler's first-DMA wait
thresholds tight, starting the write stream ~16.6 us in), while pairs
p=2..7 ship as full [128, 4352] tiles (1.09 MB DMAs for best sustained
HBM write bandwidth, ~400 GB/s measured).  Sub-diagonal lanes inside a
rectangle are shipped as garbage and discarded by the host (the mirror
of the transposed upper triangle supplies those entries).

Engine assignment is calibrated to measured TRN2 op costs: DVE
tensor_scalar (4x uop, ~165 ns overhead + 0.26 ns/col) takes the wide
segments plus fused block-B adds for p=3..5, ScalarE ACT-with-bias
(~187 + 0.83/col) the mid segments plus per-v block-B for p=0,6,7, and
GpSimd only the two narrowest fused block-B pairs (its broadcast
tensor_tensor steals SBUF bandwidth from concurrent DVE ops, so its
share is kept small).  GpSimd tensor_scalar is never used (7x slower
than its tensor_tensor).  Fused ops add the RS16 row column via
stride-0/stride-1 broadcast APs over two v rows at once.
"""

import os
import sys

for _p in ("/opt/trn_rl_repo", "/root/.axon_site/_ro/trn_rl_repo"):
    if os.path.isdir(_p) and _p not in sys.path:
        sys.path.insert(0, _p)

import numpy as np
import ml_dtypes

import concourse.bass as bass
import concourse.bacc as bacc
import concourse.mybir as mybir
import concourse.tile as tile
from concourse.bass_utils import run_bass_kernel_spmd

N = 512
CH = 256          # n_out
NIN = 256         # n_in
NCORES = 8
NB = 16           # column blocks of 32
RPB = 4           # rows per block per core
F32 = mybir.dt.float32
BF16 = mybir.dt.bfloat16
BF16NP = ml_dtypes.bfloat16

SEG = 1088        # per-v columns: 2*(w1 + w2), w1 + w2 = 544
HSLABW = 2 * SEG  # 2176, half-slab = two v rows
WXW = 2 * CH + 64 + 4  # w12 half | xsel half | b bits = 580


def _rows_for_core(k: int) -> np.ndarray:
    """Row m = 4*b + v owns global row 32*b + 4*k + v."""
    b = np.repeat(np.arange(NB), RPB)
    v = np.tile(np.arange(RPB), NB)
    return 32 * b + RPB * k + v


# --------------------------------------------------------------------------
# device program
# --------------------------------------------------------------------------

_PROGRAM = None


def _build_program() -> bass.Bass:
    nc = bacc.Bacc()
    ADD = mybir.AluOpType.add

    # wx[h]: [w12t[128h:128h+128] | xselt[128h:..] | bbits (h=0)]
    wx = nc.dram_tensor("wx", [2, 128, WXW], BF16, kind="ExternalInput")
    xt = nc.dram_tensor("xt", [NIN, N], BF16, kind="ExternalInput")
    # hslab[2p + hf]: blocks (p, 15-p), v rows {2hf, 2hf+1}; per local
    # u in [0,2): [A h0 (w1) | A h1 (w1) | B h0 (w2) | B h1 (w2)] at
    # offset 1088*u, w1 = 512-32p, w2 = 32+32p.
    hslab = nc.dram_tensor("hslab", [16, 128, HSLABW], BF16,
                           kind="ExternalOutput")

    with tile.TileContext(nc) as tc:
        with (
            tc.tile_pool(name="const", bufs=1) as cpool,
            tc.tile_pool(name="psB", bufs=2, space="PSUM") as psB,
            tc.tile_pool(name="psR", bufs=2, space="PSUM") as psR,
            tc.tile_pool(name="slabs", bufs=8) as spool,
        ):
            # ---- input loads: four DMAs on sync, matmul deps first -------
            WX, XT = [], []
            for h in range(2):
                t = cpool.tile([128, WXW], BF16, tag=f"wx{h}")
                nc.sync.dma_start(out=t[:], in_=wx[h])
                WX.append(t)
            for h in range(2):
                t = cpool.tile([128, N], BF16, tag=f"xt{h}")
                nc.sync.dma_start(out=t[:], in_=xt[128 * h:128 * (h + 1), :])
                XT.append(t)
            XS = [WX[h][:, 2 * CH:2 * CH + 64] for h in range(2)]
            bc = WX[0][:, 2 * CH + 64:WXW].bitcast(F32)  # [128, 2] f32

            mm = nc.tensor.matmul

            # ---- row terms RS[c, m] = A[row_m, c], two halves ------------
            RS = [None, None]    # f32, scalar operands for TS / ACT bias
            RS16 = [None, None]  # bf16, in1 for fused tensor_tensor
            for h in range(2):
                pr = psR.tile([128, 64], F32, tag="pr", name=f"pr{h}")
                lo = 128 * h  # W1.T columns
                mm(pr[:], WX[0][:, lo:lo + 128], XS[0],
                   start=True, stop=False)
                mm(pr[:], WX[1][:, lo:lo + 128], XS[1],
                   start=False, stop=True)
                rs = cpool.tile([128, 64], F32, tag=f"RS{h}")
                rs16 = cpool.tile([128, 64], BF16, tag=f"RS16{h}")
                nc.vector.tensor_copy(out=rs[:], in_=pr[:])
                nc.scalar.copy(out=rs16[:], in_=pr[:])
                RS[h] = rs
                RS16[h] = rs16

            # ---- column tables B'T[c, j] = B[j, c] + b[c], two halves ----
            BT = [None, None]
            for h in range(2):
                pb = psB.tile([128, N], F32, tag="pb", name=f"pb{h}")
                lo = CH + 128 * h  # W2.T columns
                mm(pb[:], WX[0][:, lo:lo + 128], XT[0][:],
                   start=True, stop=False)
                mm(pb[:], WX[1][:, lo:lo + 128], XT[1][:],
                   start=False, stop=True)
                bt = cpool.tile([128, N], BF16, tag=f"BT{h}")
                if h == 0:
                    nc.vector.tensor_scalar_add(bt[:], pb[:], bc[:, 0:1])
                else:
                    nc.scalar.add(bt[:], pb[:], bc[:, 1:2])
                BT[h] = bt

            def fused_B(eng, S, p, h, w1, w2, cB, v0, nv):
                """One op for block-B half h over nv rows starting at v0."""
                sfull = S[:]
                out = bass.AP(sfull.tensor, sfull.offset + 2 * w1 + h * w2,
                              [sfull.ap[0], [SEG, nv], [1, w2]])
                btf = BT[h][:]
                in0 = bass.AP(btf.tensor, btf.offset + cB,
                              [btf.ap[0], [0, nv], [1, w2]])
                rsf = RS16[h][:]
                in1 = bass.AP(rsf.tensor,
                              rsf.offset + RPB * (15 - p) + v0,
                              [rsf.ap[0], [1, nv], [0, w2]])
                eng.tensor_tensor(out=out, in0=in0, in1=in1, op=ADD)

            # ---- main loop: p<=1 ship as two half-slabs (fast, tightly
            # tracked stream start), p>=2 as one full slab (DMA efficiency).
            for p in range(8):
                w1 = N - 32 * p          # block p rect width (cols 32p..512)
                w2 = 32 + 32 * p         # block 15-p width
                cA = 32 * p              # B'T col offset for block p
                cB = N - w2              # for block 15-p
                halves = 4 if p == 0 else (2 if p == 1 else 1)
                nv = RPB // halves
                for hf in range(halves):
                    S = spool.tile([128, nv * SEG], BF16,
                                   tag=("sh" if p <= 1 else "sf"),
                                   name=f"s{p}_{hf}")
                    for u in range(nv):
                        v = nv * hf + u
                        off = SEG * u
                        mA = RPB * p + v
                        mB = RPB * (15 - p) + v
                        sA0 = S[:, off:off + w1]
                        sA1 = S[:, off + w1:off + 2 * w1]
                        nc.vector.tensor_scalar_add(
                            sA0, BT[0][:, cA:N], RS[0][:, mA:mA + 1])
                        if p == 0 or p >= 4:
                            nc.vector.tensor_scalar_add(
                                sA1, BT[1][:, cA:N], RS[1][:, mA:mA + 1])
                        else:
                            nc.scalar.add(sA1, BT[1][:, cA:N],
                                          RS[1][:, mA:mA + 1])
                        if p == 0 or p >= 6:
                            for h in range(2):
                                sB = S[:, off + 2 * w1 + h * w2:
                                       off + 2 * w1 + (h + 1) * w2]
                                nc.scalar.add(sB, BT[h][:, cB:N],
                                              RS[h][:, mB:mB + 1])
                    if 1 <= p <= 2:
                        fused_B(nc.gpsimd, S, p, 0, w1, w2, cB, nv * hf, nv)
                        fused_B(nc.gpsimd, S, p, 1, w1, w2, cB, nv * hf, nv)
                    elif 3 <= p <= 5:
                        fused_B(nc.vector, S, p, 0, w1, w2, cB, nv * hf, nv)
                        fused_B(nc.vector, S, p, 1, w1, w2, cB, nv * hf, nv)
                    if p == 0:
                        hq = hslab[hf // 2]
                        out_ap = bass.AP(
                            hq.tensor, hq.offset + (hf % 2) * SEG,
                            [hq.ap[0], [1, SEG]])
                        nc.sync.dma_start(out=out_ap, in_=S[:])
                    elif p == 1:
                        nc.sync.dma_start(out=hslab[2 * p + hf], in_=S[:])
                    else:
                        h0 = hslab[2 * p]
                        out_ap = bass.AP(
                            h0.tensor, h0.offset,
                            [h0.ap[0], [128 * HSLABW, 2], [1, HSLABW]])
                        nc.sync.dma_start(out=out_ap, in_=S[:])

    nc.compile()
    return nc


def _program() -> bass.Bass:
    global _PROGRAM
    if _PROGRAM is None:
        _PROGRAM = _build_program()
    return _PROGRAM


# --------------------------------------------------------------------------
# host entry point
# --------------------------------------------------------------------------

def build_in_maps(x, W, b):
    x = np.asarray(x, np.float32)
    W = np.asarray(W, np.float32)
    b = np.asarray(b, np.float32)
    w12 = np.concatenate(
        [np.ascontiguousarray(W[:, :NIN].T),
         np.ascontiguousarray(W[:, NIN:].T)], axis=1)  # [in, 512]
    xtf = np.ascontiguousarray(x.T).astype(BF16NP)
    w12 = w12.astype(BF16NP)
    bbits = np.ascontiguousarray(
        np.stack([b[0:128], b[128:256]], axis=1)).view(BF16NP)  # [128, 4]
    maps = []
    for k in range(NCORES):
        rows = _rows_for_core(k)
        xsel = np.ascontiguousarray(x[rows].T).astype(BF16NP)
        wx = np.zeros((2, 128, WXW), BF16NP)
        for h in range(2):
            lo = 128 * h
            wx[h, :, 0:2 * CH] = w12[lo:lo + 128]
            wx[h, :, 2 * CH:2 * CH + 64] = xsel[lo:lo + 128]
        wx[0, :, 2 * CH + 64:WXW] = bbits
        maps.append({"wx": wx, "xt": xtf})
    return maps


def _assemble(results):
    """8 per-core half-slab dicts -> full [512, 512, 256] f32 output."""
    out = np.zeros((N, N, CH), np.float32)
    ar = np.arange(2)
    for k in range(NCORES):
        hs = np.asarray(results[k]["hslab"]).astype(np.float32)
        for p in range(8):
            w1 = N - 32 * p
            w2 = 32 + 32 * p
            for hf in range(2):
                sp = hs[2 * p + hf].reshape(128, 2, SEG)
                rowsA = 32 * p + RPB * k + 2 * hf + ar
                rowsB = 32 * (15 - p) + RPB * k + 2 * hf + ar
                out[rowsA, 32 * p:N, 0:128] = \
                    sp[:, :, 0:w1].transpose(1, 2, 0)
                out[rowsA, 32 * p:N, 128:256] = \
                    sp[:, :, w1:2 * w1].transpose(1, 2, 0)
                out[rowsB, N - w2:N, 0:128] = \
                    sp[:, :, 2 * w1:2 * w1 + w2].transpose(1, 2, 0)
                out[rowsB, N - w2:N, 128:256] = \
                    sp[:, :, 2 * w1 + w2:SEG].transpose(1, 2, 0)
    # unshard: keep the strict upper triangle (sub-diagonal rect lanes are
    # garbage), mirror it across the diagonal; diag stays scatter-init 0.
    tril = np.tril_indices(N)
    out[tril] = 0.0
    return out + out.transpose(1, 0, 2)


def kernel(x, W, b):
    nc = _program()
    in_maps = build_in_maps(x, W, b)
    res = run_bass_kernel_spmd(nc, in_maps, core_ids=list(range(NCORES)))
    return _assemble(res.results)
